# revision 1
# baseline (speedup 1.0000x reference)
"""Trainium2 Bass kernel for nn_AutoRegressiveDistribution (MADE sampling).

Self-contained: hardcodes shapes/sharding. Shards batch B across 8 cores,
runs the D-step autoregressive sampling loop fully on-device per core.

Per-core structure (rows = S*BS = 512, processed as TWO independent
half-chains of an s-pair each so the serial per-step dependency chains
overlap across engines):
  - Hidden units are permuted host-side, sorted by MADE degree (1..63),
    each degree block placed at a 32-aligned slot of a padded unit space
    (matmul operands must start at partition 0/32/64). At step i only the
    block of degree i gets a fresh pre-activation (depends on z_{<i}),
    is relu'd (to bf16), and contributes into a persistent PSUM
    accumulator OUT (batch on partitions, (s, outcol) on free).
  - z-history kept in batch-on-partitions (cheap vector ops) and
    degree-on-partitions zT (matmul rhs), bridged per step by full PE
    re-transposes (float32r) + a 32-row-group PSUM->SBUF copy.
  - The z-path matmuls use float32r (fp32 bits, full-rate for N>=256);
    the output contributions use bf16.
  - ctx_h = Wc @ ctx + b1 is precomputed once; bout is seeded into OUT
    once via ones-outer-product matmuls, so the per-step chain is
    hist-MM -> relu -> contrib -> exp -> ln -> mul/add -> transpose ->
    copy with no bias work.
"""

import numpy as np
from contextlib import ExitStack

import concourse.bass as bass
import concourse.tile as tile
from concourse import bacc, mybir
from concourse.bass_utils import run_bass_kernel_spmd

D, H, CTX, B, S = 64, 1024, 256, 1024, 4
NCORES = 8
BS = B // NCORES          # 128 batch rows per core
R = S * BS                # 512 rows per core
RH = R // 2               # rows per half-chain (s-pair)
FP32 = mybir.dt.float32
BF16 = mybir.dt.bfloat16
F32R = mybir.dt.float32r

HP = 2048  # padded hidden units: degree block i at [32*(i-1), 32*(i-1)+cnt[i])


def _made_struct():
    mh = (np.arange(H) % (D - 1)) + 1            # degrees 1..63
    perm = np.argsort(mh, kind="stable")
    mh_s = mh[perm]
    cnt = np.bincount(mh_s, minlength=D)          # cnt[d] = #units of degree d
    off = np.concatenate([[0], np.cumsum(cnt)[:-1]]).astype(np.int64)
    return mh, perm, mh_s, cnt, off


def _prep_weights(W1, b1, Wc, Wout):
    """Mask + permute + 32-pad weights host-side (cheap, O(weight size))."""
    mh, perm, mh_s, cnt, off = _made_struct()
    m0 = np.arange(1, D + 1)
    M1 = (mh[:, None] >= m0[None, :]).astype(np.float32)          # (H, D)
    mout = np.concatenate([m0, m0])                                # (2D,)
    Mout = (mout[:, None] > mh[None, :]).astype(np.float32)        # (2D, H)
    W1m = (W1 * M1)[perm]                   # (H, D) permuted rows
    Woutm = (Wout * Mout)[:, perm]          # (2D, H) permuted cols
    src = np.arange(H)
    pdst = 32 * (mh_s - 1) + (src - off[mh_s])   # padded slot of sorted unit
    import ml_dtypes
    bf = ml_dtypes.bfloat16
    W1T = np.zeros((D, HP), np.float32)
    W1T[:, pdst] = W1m.T
    WcT = np.zeros((CTX, HP), np.float32)
    WcT[:, pdst] = Wc[perm].T
    b1p = np.zeros((HP, 1), np.float32)
    b1p[pdst, 0] = b1[perm]
    WoutB = np.zeros((32, D - 1, 2 * D), np.float32)  # (slot, block, outcol)
    WoutB[pdst % 32, (mh_s - 1)] = Woutm[:, :].T[src]
    return W1T, WoutB, WcT, b1p


_PROGRAM_CACHE = None


def _pin_act_table():
    """Make Exp/Ln/Relu resolvable only via natural_log_exp_and_others so
    the act-table chooser doesn't thrash between the exp and ln tables
    (each LoadActFuncSet costs ~1.3us). Table positions are preserved so
    act_func_set_id stays consistent with act_info.json."""
    import concourse.bacc as bacc_mod
    from concourse import hw_specs
    orig = hw_specs.get_activation_tables
    AF = mybir.ActivationFunctionType
    pin = {AF.Exp, AF.Ln, AF.Relu}

    def filtered(arch):
        out = {}
        for name, fns in orig(arch).items():
            if name == "natural_log_exp_and_others":
                out[name] = set(fns)
            else:
                out[name] = set(fns) - pin
        return out

    bacc_mod.get_activation_tables = filtered


def _build_program():
    """Build + compile the SPMD Bass program (input-independent, cached)."""
    global _PROGRAM_CACHE
    if _PROGRAM_CACHE is not None:
        return _PROGRAM_CACHE
    _pin_act_table()
    _, _, mh_s, cnt, off = _made_struct()

    nc = bacc.Bacc("TRN2", target_bir_lowering=False, debug=False,
                   num_devices=NCORES)

    ctx_d = nc.dram_tensor("ctx", (BS, CTX), FP32, kind="ExternalInput")
    eps_d = nc.dram_tensor("eps", (S, BS, D), FP32, kind="ExternalInput")
    w1t_d = nc.dram_tensor("w1t", (D, HP), FP32, kind="ExternalInput")
    woutt_d = nc.dram_tensor("woutt", (32, D - 1, 2 * D), FP32,
                             kind="ExternalInput")
    wct_d = nc.dram_tensor("wct", (CTX, HP), FP32, kind="ExternalInput")
    b1_d = nc.dram_tensor("b1", (HP, 1), FP32, kind="ExternalInput")
    boutb_d = nc.dram_tensor("boutb", (128, 2 * D), FP32, kind="ExternalInput")
    ident_d = nc.dram_tensor("ident", (128, 128), FP32, kind="ExternalInput")
    z_d = nc.dram_tensor("z_out", (S, BS, D), FP32, kind="ExternalOutput")
    mu_d = nc.dram_tensor("mu_out", (S, BS, D), FP32, kind="ExternalOutput")
    sc_d = nc.dram_tensor("sc_out", (S, BS, D), FP32, kind="ExternalOutput")

    AF = mybir.ActivationFunctionType
    OP = mybir.AluOpType

    with tile.TileContext(nc) as tc, ExitStack() as ctx:
        singles = ctx.enter_context(tc.tile_pool(name="singles", bufs=1))
        ablk_pool = ctx.enter_context(tc.tile_pool(name="ablk", bufs=3))
        scratch = ctx.enter_context(tc.tile_pool(name="scratch", bufs=3))
        psA = ctx.enter_context(tc.tile_pool(name="psA", bufs=2, space="PSUM"))
        psOut = ctx.enter_context(tc.tile_pool(name="psOut", bufs=1,
                                               space="PSUM"))
        psZ = ctx.enter_context(tc.tile_pool(name="psZ", bufs=1, space="PSUM"))

        # ---- load inputs/constants into SBUF ----
        ctx_sb = singles.tile([BS, CTX], FP32)
        nc.sync.dma_start(ctx_sb[:], ctx_d.ap())
        w1t_sb = singles.tile([D, HP], FP32)
        nc.sync.dma_start(w1t_sb[:], w1t_d.ap())
        woutt_sb = singles.tile([32, D - 1, 2 * D], FP32)
        nc.sync.dma_start(woutt_sb[:], woutt_d.ap())
        wct_sb = singles.tile([128, 2, HP], FP32)
        nc.sync.dma_start(wct_sb[:],
                          wct_d.ap().rearrange("(k p) h -> p k h", p=128))
        b1_sb = singles.tile([128, HP // 128], FP32)
        nc.sync.dma_start(b1_sb[:],
                          b1_d.ap().rearrange("(c p) one -> p (c one)", p=128))
        boutb_sb = singles.tile([128, 2 * D], FP32)
        nc.sync.dma_start(boutb_sb[:], boutb_d.ap())
        boutbb_sb = singles.tile([1, 2 * D], BF16)
        nc.vector.tensor_copy(boutbb_sb[:], boutb_sb[0:1, :])
        ident_sb = singles.tile([128, 128], FP32)
        nc.sync.dma_start(ident_sb[:], ident_d.ap())
        ones_sb = singles.tile([1, 128], FP32)
        nc.vector.memset(ones_sb[:], 1.0)

        eps2 = [singles.tile([BS, 2, D], FP32, tag=f"eps{h}", name=f"eps{h}")
                for h in (0, 1)]
        for h in (0, 1):
            nc.sync.dma_start(
                eps2[h][:],
                eps_d.ap()[2 * h:2 * h + 2].rearrange("s b d -> b s d"))

        # ---- ctxT: (BS, CTX) -> (CTX, BS) in 2 chunks ----
        ctxT_sb = singles.tile([128, 2, BS], FP32)
        for k in range(2):
            ps = psA.tile([128, BS], FP32, tag="aps0")
            nc.tensor.transpose(ps[:], ctx_sb[:, k * 128:(k + 1) * 128],
                                ident_sb[:])
            nc.vector.tensor_copy(ctxT_sb[:, k, :], ps[:])

        # ---- A_base = WcT.T @ ctxT + b1 : (HP, BS) in 16 unit-chunks ----
        NCH = HP // 128
        a_base = singles.tile([128, NCH, BS], FP32)
        for hc in range(NCH):
            ps = psA.tile([128, BS], FP32, tag="aps0")
            for k in range(2):
                nc.tensor.matmul(
                    ps[:],
                    wct_sb[:, k, hc * 128:(hc + 1) * 128],
                    ctxT_sb[:, k, :],
                    start=(k == 0), stop=(k == 1))
            nc.vector.tensor_scalar_add(a_base[:, hc, :], ps[:],
                                        b1_sb[:, hc:hc + 1])

        # ---- per-half state ----
        z2 = [singles.tile([BS, 2, D], FP32, tag=f"z{h}", name=f"z{h}")
              for h in (0, 1)]
        mu2 = [singles.tile([BS, 2, D], FP32, tag=f"mu{h}", name=f"mu{h}")
               for h in (0, 1)]
        sc2 = [singles.tile([BS, 2, D], FP32, tag=f"sc{h}", name=f"sc{h}")
               for h in (0, 1)]
        zT2 = [singles.tile([D, RH], FP32, tag=f"zT{h}", name=f"zT{h}")
               for h in (0, 1)]
        outr2 = [psOut.tile([128, 2, 128], FP32, tag=f"outr{h}",
                              name=f"outr{h}") for h in (0, 1)]
        zTps2 = [psZ.tile([D, RH], FP32, tag=f"zTps{h}", name=f"zTps{h}")
                 for h in (0, 1)]

        for h in (0, 1):
            nc.vector.memset(z2[h][:], 0.0)

        def retranspose(i, h):
            """Re-transpose Z half h (cols > i garbage, rows > i of zT never
            read before refresh); copy row-group of row i psum->sbuf."""
            for s in (0, 1):
                nc.tensor.transpose(
                    zTps2[h][:, s * BS:(s + 1) * BS],
                    z2[h][:, s, :],
                    ident_sb[:])
            g = 32 * (i // 32)
            nc.vector.tensor_copy(zT2[h][g:g + 32, :], zTps2[h][g:g + 32, :])

        # ---- step 0 (bias-only): mu0 = bout[0], sc0 = softplus(bout[D]) ----
        for h in (0, 1):
            sp_tmp = scratch.tile([BS, 2], FP32, tag=f"sp{h}")
            nc.vector.tensor_copy(mu2[h][:, :, 0],
                                  boutb_sb[:, 0:1].to_broadcast((BS, 2)))
            nc.scalar.activation(out=sp_tmp[:],
                                 in_=boutb_sb[:, D:D + 1].to_broadcast((BS, 2)),
                                 func=AF.Exp, bias=0.0, scale=1.0)
            nc.scalar.activation(out=sc2[h][:, :, 0], in_=sp_tmp[:],
                                 func=AF.Ln, bias=1.0, scale=1.0)
            nc.vector.tensor_scalar(out=z2[h][:, :, 0], in0=eps2[h][:, :, 0],
                                    scalar1=sc2[h][:, 0, 0:1],
                                    scalar2=boutb_sb[:, 0:1],
                                    op0=OP.mult, op1=OP.add)
            retranspose(0, h)

        # ---- steps 1..63, two interleaved half-chains ----
        for i in range(1, D):
            nn = int(cnt[i])
            pp = 32 * (i - 1)              # padded unit offset of block i
            c, pl = pp // 128, pp % 128
            q = min(pl, 64)                # 32-aligned base (96 -> 64)
            kk = pl - q + nn
            for h in (0, 1):
                # fresh block pre-activation = ctx_base (shifted-identity MM)
                # + W1T[0:i, blk].T @ zT[0:i]
                aps_t = psA.tile([nn, RH], FP32, tag=f"aps{h}")
                sl = a_base[q:q + kk, c, :]
                rhs = bass.AP(sl.tensor, sl.offset,
                              [sl.ap[0], [0, 2], sl.ap[-1]])
                nc.tensor.matmul(aps_t[:],
                                 ident_sb[q:q + kk, pl:pl + nn],
                                 rhs, start=True, stop=False)
                nc.tensor.matmul(aps_t[:],
                                 w1t_sb[0:i, pp:pp + nn],
                                 zT2[h][0:i, :],
                                 start=False, stop=True)
                # relu -> bf16 (alternate engines across halves)
                ab = ablk_pool.tile([nn, RH], FP32, tag=f"ablk{h}")
                if h == 0:
                    nc.vector.tensor_scalar_max(ab[:], aps_t[:], 0.0)
                else:
                    nc.scalar.activation(out=ab[:], in_=aps_t[:],
                                         func=AF.Relu, bias=0.0, scale=1.0)

                if i == 1:
                    # seed OUT with bout once (ones ⊗ bout row)
                    for s in (0, 1):
                        nc.tensor.matmul(outr2[h][:, s, :], ones_sb[:],
                                         boutb_sb[0:1, :],
                                         start=(s == 0), stop=False,
                                         skip_group_check=True)

                # contribution: OUT[:, s, cols >= i] += ab_s.T @ WoutB[blk]
                # (cols < i are never read again -> halve the stream)
                for s in (0, 1):
                    for c0, c1 in ((i, D), (D + i, 2 * D)):
                        nc.tensor.matmul(outr2[h][:, s, c0:c1],
                                         ab[:, s * BS:(s + 1) * BS],
                                         woutt_sb[0:nn, i - 1, c0:c1],
                                         start=False,
                                         stop=(i == D - 1 and s == 1
                                               and c0 >= D),
                                         skip_group_check=True)

                # z-step: sc = softplus(OUT[ps]), z = OUT[mu] + sc*eps
                sp_tmp = scratch.tile([BS, 2], FP32, tag=f"sp{h}")
                nc.scalar.activation(out=sp_tmp[:], in_=outr2[h][:, :, D + i],
                                     func=AF.Exp, bias=0.0, scale=1.0)
                nc.scalar.activation(out=sc2[h][:, :, i], in_=sp_tmp[:],
                                     func=AF.Ln, bias=1.0, scale=1.0)
                tse = scratch.tile([BS, 2], FP32, tag=f"tse{h}")
                nc.vector.tensor_mul(tse[:], sc2[h][:, :, i], eps2[h][:, :, i])
                nc.vector.tensor_add(z2[h][:, :, i], tse[:],
                                     outr2[h][:, :, i])
                if i < D - 1:
                    retranspose(i, h)

        # ---- mu extraction (batched) + outputs ----
        for h in (0, 1):
            nc.vector.tensor_copy(mu2[h][:, :, 1:D], outr2[h][:, :, 1:D])
            nc.sync.dma_start(
                z_d.ap()[2 * h:2 * h + 2].rearrange("s b d -> b s d"),
                z2[h][:])
            nc.sync.dma_start(
                mu_d.ap()[2 * h:2 * h + 2].rearrange("s b d -> b s d"),
                mu2[h][:])
            nc.sync.dma_start(
                sc_d.ap()[2 * h:2 * h + 2].rearrange("s b d -> b s d"),
                sc2[h][:])

    nc.compile()
    _PROGRAM_CACHE = nc
    return nc


def _in_maps(context, eps, W1, b1, Wc, Wout, bout):
    W1T, WoutB, WcT, b1p = _prep_weights(W1, b1, Wc, Wout)
    ident = np.eye(128, dtype=np.float32)
    boutb = np.ascontiguousarray(np.tile(bout.reshape(1, -1), (128, 1)))
    maps = []
    for c in range(NCORES):
        maps.append({
            "ctx": np.ascontiguousarray(context[c * BS:(c + 1) * BS]),
            "eps": np.ascontiguousarray(eps[:, c * BS:(c + 1) * BS]),
            "w1t": W1T, "woutt": WoutB, "wct": WcT, "b1": b1p,
            "boutb": boutb, "ident": ident,
        })
    return maps


def run(context, eps, W1, b1, Wc, Wout, bout, trace=False):
    context = np.asarray(context, np.float32)
    eps = np.asarray(eps, np.float32)
    W1 = np.asarray(W1, np.float32)
    b1 = np.asarray(b1, np.float32)
    Wc = np.asarray(Wc, np.float32)
    Wout = np.asarray(Wout, np.float32)
    bout = np.asarray(bout, np.float32)
    nc = _build_program()
    maps = _in_maps(context, eps, W1, b1, Wc, Wout, bout)
    res = run_bass_kernel_spmd(nc, maps, core_ids=list(range(NCORES)),
                               trace=trace)
    z = np.empty((S, B, D), np.float32)
    mu = np.empty((S, B, D), np.float32)
    sc = np.empty((S, B, D), np.float32)
    for c in range(NCORES):
        z[:, c * BS:(c + 1) * BS] = res.results[c]["z_out"]
        mu[:, c * BS:(c + 1) * BS] = res.results[c]["mu_out"]
        sc[:, c * BS:(c + 1) * BS] = res.results[c]["sc_out"]
    return (z, mu, sc), res


def kernel(context, eps, W1, b1, Wc, Wout, bout):
    (z, mu, sc), _ = run(context, eps, W1, b1, Wc, Wout, bout)
    return z, mu, sc



# revision 6
# speedup vs baseline: 1.2624x; 1.2624x over previous
"""Trainium2 Bass kernel for nn_AutoRegressiveDistribution (MADE sampling).

Self-contained: hardcodes shapes/sharding. Shards batch B across 8 cores,
runs the D-step autoregressive sampling loop fully on-device per core.

Per-core structure (rows = S*BS = 512, processed as TWO independent
half-chains of an s-pair each so the serial per-step dependency chains
overlap across engines):
  - Hidden units are permuted host-side, sorted by MADE degree (1..63),
    each degree block placed at a 32-aligned slot of a padded unit space
    (matmul operands must start at partition 0/32/64). At step i only the
    block of degree i gets a fresh pre-activation (depends on z_{<i}),
    is relu'd (to bf16), and contributes into a persistent PSUM
    accumulator OUT (batch on partitions, (s, outcol) on free).
  - z-history kept in batch-on-partitions (cheap vector ops) and
    degree-on-partitions zT (matmul rhs), bridged per step by full PE
    re-transposes + a 32-row-group PSUM->SBUF copy.
  - The z-path matmuls use real float32r tiles (DMA from f32r dram /
    DVE-rounded copies) so the cost is 1 cycle/row at N>=256 instead of
    fp32's 4 cycles/row; the output contributions use bf16 (relu writes
    bf16 directly, Wout pre-converted host-side).
  - ctx_h = Wc @ ctx + b1 is precomputed once; bout is seeded into OUT
    once via ones-outer-product matmuls, so the per-step chain is
    hist-MM -> relu -> contrib -> exp -> ln -> mul/add -> transpose ->
    copy with no bias work.
"""

import numpy as np
from contextlib import ExitStack

import concourse.bass as bass
import concourse.tile as tile
from concourse import bacc, mybir
from concourse.bass_utils import run_bass_kernel_spmd

D, H, CTX, B, S = 64, 1024, 256, 1024, 4
NCORES = 8
BS = B // NCORES          # 128 batch rows per core
R = S * BS                # 512 rows per core
RH = R // 2               # rows per half-chain (s-pair)
FP32 = mybir.dt.float32
BF16 = mybir.dt.bfloat16
F32R = mybir.dt.float32r

HP = 2048  # padded hidden units: degree block i at [32*(i-1), 32*(i-1)+cnt[i])


def _made_struct():
    mh = (np.arange(H) % (D - 1)) + 1            # degrees 1..63
    perm = np.argsort(mh, kind="stable")
    mh_s = mh[perm]
    cnt = np.bincount(mh_s, minlength=D)          # cnt[d] = #units of degree d
    off = np.concatenate([[0], np.cumsum(cnt)[:-1]]).astype(np.int64)
    return mh, perm, mh_s, cnt, off


def _prep_weights(W1, b1, Wc, Wout):
    """Mask + permute + 32-pad weights host-side (cheap, O(weight size))."""
    mh, perm, mh_s, cnt, off = _made_struct()
    m0 = np.arange(1, D + 1)
    M1 = (mh[:, None] >= m0[None, :]).astype(np.float32)          # (H, D)
    mout = np.concatenate([m0, m0])                                # (2D,)
    Mout = (mout[:, None] > mh[None, :]).astype(np.float32)        # (2D, H)
    W1m = (W1 * M1)[perm]                   # (H, D) permuted rows
    Woutm = (Wout * Mout)[:, perm]          # (2D, H) permuted cols
    src = np.arange(H)
    pdst = 32 * (mh_s - 1) + (src - off[mh_s])   # padded slot of sorted unit
    import ml_dtypes
    bf = ml_dtypes.bfloat16
    W1T = np.zeros((D, HP), np.float32)
    W1T[:, pdst] = W1m.T
    WcT = np.zeros((CTX, HP), np.float32)
    WcT[:, pdst] = Wc[perm].T
    b1p = np.zeros((HP, 1), np.float32)
    b1p[pdst, 0] = b1[perm]
    WoutB = np.zeros((32, D - 1, 2 * D), np.float32)  # (slot, block, outcol)
    WoutB[pdst % 32, (mh_s - 1)] = Woutm[:, :].T[src]
    return W1T, WoutB.astype(bf), WcT, b1p


_PROGRAM_CACHE = None


def _pin_act_table():
    """Make Exp/Ln/Relu resolvable only via natural_log_exp_and_others so
    the act-table chooser doesn't thrash between the exp and ln tables
    (each LoadActFuncSet costs ~1.3us). Table positions are preserved so
    act_func_set_id stays consistent with act_info.json."""
    import concourse.bacc as bacc_mod
    from concourse import hw_specs
    orig = hw_specs.get_activation_tables
    AF = mybir.ActivationFunctionType
    pin = {AF.Exp, AF.Ln, AF.Relu}

    def filtered(arch):
        out = {}
        for name, fns in orig(arch).items():
            if name == "natural_log_exp_and_others":
                out[name] = set(fns)
            else:
                out[name] = set(fns) - pin
        return out

    bacc_mod.get_activation_tables = filtered


def _build_program():
    """Build + compile the SPMD Bass program (input-independent, cached)."""
    global _PROGRAM_CACHE
    if _PROGRAM_CACHE is not None:
        return _PROGRAM_CACHE
    _pin_act_table()
    _, _, mh_s, cnt, off = _made_struct()

    nc = bacc.Bacc("TRN2", target_bir_lowering=False, debug=False,
                   num_devices=NCORES)

    ctx_d = nc.dram_tensor("ctx", (BS, CTX), FP32, kind="ExternalInput")
    eps_d = nc.dram_tensor("eps", (S, BS, D), FP32, kind="ExternalInput")
    w1t_d = nc.dram_tensor("w1t", (D, HP), F32R, kind="ExternalInput")
    woutt_d = nc.dram_tensor("woutt", (32, D - 1, 2 * D), BF16,
                             kind="ExternalInput")
    wct_d = nc.dram_tensor("wct", (CTX, HP), F32R, kind="ExternalInput")
    b1_d = nc.dram_tensor("b1", (HP, 1), FP32, kind="ExternalInput")
    boutb_d = nc.dram_tensor("boutb", (128, 2 * D), FP32, kind="ExternalInput")
    ident_d = nc.dram_tensor("ident", (128, 128), F32R, kind="ExternalInput")
    z_d = nc.dram_tensor("z_out", (S, BS, D), FP32, kind="ExternalOutput")
    mu_d = nc.dram_tensor("mu_out", (S, BS, D), FP32, kind="ExternalOutput")
    sc_d = nc.dram_tensor("sc_out", (S, BS, D), FP32, kind="ExternalOutput")

    AF = mybir.ActivationFunctionType
    OP = mybir.AluOpType

    with tile.TileContext(nc) as tc, ExitStack() as ctx:
        singles = ctx.enter_context(tc.tile_pool(name="singles", bufs=1))
        ablk_pool = ctx.enter_context(tc.tile_pool(name="ablk", bufs=3))
        scratch = ctx.enter_context(tc.tile_pool(name="scratch", bufs=3))
        psA = ctx.enter_context(tc.tile_pool(name="psA", bufs=2, space="PSUM"))
        psOut = ctx.enter_context(tc.tile_pool(name="psOut", bufs=1,
                                               space="PSUM"))
        psZ = ctx.enter_context(tc.tile_pool(name="psZ", bufs=1, space="PSUM"))

        # ---- load inputs/constants into SBUF ----
        ctx_sb = singles.tile([BS, CTX], FP32)
        nc.sync.dma_start(ctx_sb[:], ctx_d.ap())
        w1t_sb = singles.tile([D, HP], F32R)
        nc.sync.dma_start(w1t_sb[:], w1t_d.ap())
        woutt_sb = singles.tile([32, D - 1, 2 * D], BF16)
        nc.sync.dma_start(woutt_sb[:], woutt_d.ap())
        wct_sb = singles.tile([128, 2, HP], F32R)
        nc.sync.dma_start(wct_sb[:],
                          wct_d.ap().rearrange("(k p) h -> p k h", p=128))
        b1_sb = singles.tile([128, HP // 128], FP32)
        nc.sync.dma_start(b1_sb[:],
                          b1_d.ap().rearrange("(c p) one -> p (c one)", p=128))
        boutb_sb = singles.tile([128, 2 * D], FP32)
        nc.sync.dma_start(boutb_sb[:], boutb_d.ap())
        boutbb_sb = singles.tile([1, 2 * D], BF16)
        nc.vector.tensor_copy(boutbb_sb[:], boutb_sb[0:1, :])
        ident_sb = singles.tile([128, 128], F32R)
        nc.sync.dma_start(ident_sb[:], ident_d.ap())
        ones_sb = singles.tile([1, 128], BF16)
        nc.vector.memset(ones_sb[:], 1.0)

        eps2 = [singles.tile([BS, 2, D], FP32, tag=f"eps{h}", name=f"eps{h}")
                for h in (0, 1)]
        for h in (0, 1):
            nc.sync.dma_start(
                eps2[h][:],
                eps_d.ap()[2 * h:2 * h + 2].rearrange("s b d -> b s d"))

        # ---- ctx in f32r (rounded copy) for the transpose ----
        ctxr_sb = singles.tile([BS, CTX], F32R)
        nc.vector.tensor_copy(ctxr_sb[:], ctx_sb[:])

        # ---- ctxT: (BS, CTX) -> (CTX, BS) in 2 chunks ----
        ctxT_sb = singles.tile([128, 2, BS], F32R)
        for k in range(2):
            ps = psA.tile([128, BS], F32R, tag="aps0")
            nc.tensor.transpose(ps[:], ctxr_sb[:, k * 128:(k + 1) * 128],
                                ident_sb[:])
            nc.vector.tensor_copy(ctxT_sb[:, k, :], ps[:])

        # ---- A_base = WcT.T @ ctxT + b1 : (HP, BS) in 16 unit-chunks ----
        NCH = HP // 128
        a_base = singles.tile([128, NCH, BS], F32R)
        for hc in range(NCH):
            ps = psA.tile([128, BS], FP32, tag="aps0")
            for k in range(2):
                nc.tensor.matmul(
                    ps[:],
                    wct_sb[:, k, hc * 128:(hc + 1) * 128],
                    ctxT_sb[:, k, :],
                    start=(k == 0), stop=(k == 1))
            nc.vector.tensor_scalar_add(a_base[:, hc, :], ps[:],
                                        b1_sb[:, hc:hc + 1])

        # ---- per-half state ----
        z2 = [singles.tile([BS, 2, D], F32R, tag=f"z{h}", name=f"z{h}")
              for h in (0, 1)]
        zf2 = [singles.tile([BS, 2, D], FP32, tag=f"zf{h}", name=f"zf{h}")
               for h in (0, 1)]
        mu2 = [singles.tile([BS, 2, D], FP32, tag=f"mu{h}", name=f"mu{h}")
               for h in (0, 1)]
        sc2 = [singles.tile([BS, 2, D], FP32, tag=f"sc{h}", name=f"sc{h}")
               for h in (0, 1)]
        zT2 = [singles.tile([D, RH], F32R, tag=f"zT{h}", name=f"zT{h}")
               for h in (0, 1)]
        outr2 = [psOut.tile([128, 2, 128], FP32, tag=f"outr{h}",
                              name=f"outr{h}") for h in (0, 1)]
        zTps2 = [psZ.tile([D, RH], F32R, tag=f"zTps{h}", name=f"zTps{h}")
                 for h in (0, 1)]

        for h in (0, 1):
            nc.vector.memset(z2[h][:].bitcast(FP32), 0.0)

        def retranspose(i, h):
            """Re-transpose Z half h (cols > i garbage, rows > i of zT never
            read before refresh); copy row-group of row i psum->sbuf."""
            for s in (0, 1):
                nc.tensor.transpose(
                    zTps2[h][:, s * BS:(s + 1) * BS],
                    z2[h][:, s, :],
                    ident_sb[:])
            g = 32 * (i // 32)
            nc.vector.tensor_copy(zT2[h][g:g + 32, :], zTps2[h][g:g + 32, :])

        # ---- step 0 (bias-only): mu0 = bout[0], sc0 = softplus(bout[D]) ----
        for h in (0, 1):
            sp_tmp = scratch.tile([BS, 2], FP32, tag=f"sp{h}")
            nc.vector.tensor_copy(mu2[h][:, :, 0],
                                  boutb_sb[:, 0:1].to_broadcast((BS, 2)))
            nc.scalar.activation(out=sp_tmp[:],
                                 in_=boutb_sb[:, D:D + 1].to_broadcast((BS, 2)),
                                 func=AF.Exp, bias=0.0, scale=1.0)
            nc.scalar.activation(out=sc2[h][:, :, 0], in_=sp_tmp[:],
                                 func=AF.Ln, bias=1.0, scale=1.0)
            nc.vector.tensor_scalar(out=z2[h][:, :, 0], in0=eps2[h][:, :, 0],
                                    scalar1=sc2[h][:, 0, 0:1],
                                    scalar2=boutb_sb[:, 0:1],
                                    op0=OP.mult, op1=OP.add)
            retranspose(0, h)

        # ---- steps 1..63, two interleaved half-chains ----
        for i in range(1, D):
            nn = int(cnt[i])
            pp = 32 * (i - 1)              # padded unit offset of block i
            c, pl = pp // 128, pp % 128
            q = min(pl, 64)                # 32-aligned base (96 -> 64)
            kk = pl - q + nn
            for h in (0, 1):
                # fresh block pre-activation = ctx_base (shifted-identity MM)
                # + W1T[0:i, blk].T @ zT[0:i]
                aps_t = psA.tile([nn, RH], FP32, tag=f"aps{h}")
                sl = a_base[q:q + kk, c, :]
                rhs = bass.AP(sl.tensor, sl.offset,
                              [sl.ap[0], [0, 2], sl.ap[-1]])
                nc.tensor.matmul(aps_t[:],
                                 ident_sb[q:q + kk, pl:pl + nn],
                                 rhs, start=True, stop=False)
                nc.tensor.matmul(aps_t[:],
                                 w1t_sb[0:i, pp:pp + nn],
                                 zT2[h][0:i, :],
                                 start=False, stop=True)
                # relu -> bf16 (alternate engines across halves)
                ab = ablk_pool.tile([nn, RH], BF16, tag=f"ablk{h}")
                if h == 0:
                    nc.vector.tensor_scalar_max(ab[:], aps_t[:], 0.0)
                else:
                    nc.scalar.activation(out=ab[:], in_=aps_t[:],
                                         func=AF.Relu, bias=0.0, scale=1.0)

                if i == 1:
                    # seed OUT with bout once (ones ⊗ bout row)
                    for s in (0, 1):
                        nc.tensor.matmul(outr2[h][:, s, :], ones_sb[:],
                                         boutbb_sb[:],
                                         start=(s == 0), stop=False,
                                         skip_group_check=True)

                # contribution: OUT[:, s, cols {i..D} u {D+i..2D}] += ab_s.T
                # @ WoutB[blk] (cols < i never read again); both col ranges
                # in one strided MM per s.
                for s in (0, 1):
                    wsl = woutt_sb[0:nn, i - 1, :]
                    wap = bass.AP(wsl.tensor, wsl.offset + i,
                                  [wsl.ap[0], [D, 2], [1, D - i]])
                    osl = outr2[h][:, s, :]
                    oap = bass.AP(osl.tensor, osl.offset + i,
                                  [osl.ap[0], [D, 2], [1, D - i]])
                    nc.tensor.matmul(oap,
                                     ab[:, s * BS:(s + 1) * BS],
                                     wap,
                                     start=False,
                                     stop=(i == D - 1 and s == 1),
                                     skip_group_check=True)

                # z-step: sc = softplus(OUT[ps]), z = OUT[mu] + sc*eps
                sp_tmp = scratch.tile([BS, 2], FP32, tag=f"sp{h}")
                nc.scalar.activation(out=sp_tmp[:], in_=outr2[h][:, :, D + i],
                                     func=AF.Exp, bias=0.0, scale=1.0)
                nc.scalar.activation(out=sc2[h][:, :, i], in_=sp_tmp[:],
                                     func=AF.Ln, bias=1.0, scale=1.0)
                tse = scratch.tile([BS, 2], FP32, tag=f"tse{h}")
                nc.vector.tensor_mul(tse[:], sc2[h][:, :, i], eps2[h][:, :, i])
                nc.vector.tensor_add(z2[h][:, :, i], tse[:],
                                     outr2[h][:, :, i])
                if i < D - 1:
                    retranspose(i, h)

        # ---- mu extraction (batched) + outputs ----
        for h in (0, 1):
            nc.vector.tensor_copy(mu2[h][:, :, 1:D], outr2[h][:, :, 1:D])
            nc.vector.tensor_copy(zf2[h][:], z2[h][:])
            nc.sync.dma_start(
                z_d.ap()[2 * h:2 * h + 2].rearrange("s b d -> b s d"),
                zf2[h][:])
            nc.sync.dma_start(
                mu_d.ap()[2 * h:2 * h + 2].rearrange("s b d -> b s d"),
                mu2[h][:])
            nc.sync.dma_start(
                sc_d.ap()[2 * h:2 * h + 2].rearrange("s b d -> b s d"),
                sc2[h][:])

    nc.compile()
    _PROGRAM_CACHE = nc
    return nc


def _in_maps(context, eps, W1, b1, Wc, Wout, bout):
    W1T, WoutB, WcT, b1p = _prep_weights(W1, b1, Wc, Wout)
    ident = np.eye(128, dtype=np.float32)
    boutb = np.ascontiguousarray(np.tile(bout.reshape(1, -1), (128, 1)))
    maps = []
    for c in range(NCORES):
        maps.append({
            "ctx": np.ascontiguousarray(context[c * BS:(c + 1) * BS]),
            "eps": np.ascontiguousarray(eps[:, c * BS:(c + 1) * BS]),
            "w1t": W1T, "woutt": WoutB, "wct": WcT, "b1": b1p,
            "boutb": boutb, "ident": ident,
        })
    return maps


def run(context, eps, W1, b1, Wc, Wout, bout, trace=False):
    context = np.asarray(context, np.float32)
    eps = np.asarray(eps, np.float32)
    W1 = np.asarray(W1, np.float32)
    b1 = np.asarray(b1, np.float32)
    Wc = np.asarray(Wc, np.float32)
    Wout = np.asarray(Wout, np.float32)
    bout = np.asarray(bout, np.float32)
    nc = _build_program()
    maps = _in_maps(context, eps, W1, b1, Wc, Wout, bout)
    res = run_bass_kernel_spmd(nc, maps, core_ids=list(range(NCORES)),
                               trace=trace)
    z = np.empty((S, B, D), np.float32)
    mu = np.empty((S, B, D), np.float32)
    sc = np.empty((S, B, D), np.float32)
    for c in range(NCORES):
        z[:, c * BS:(c + 1) * BS] = res.results[c]["z_out"]
        mu[:, c * BS:(c + 1) * BS] = res.results[c]["mu_out"]
        sc[:, c * BS:(c + 1) * BS] = res.results[c]["sc_out"]
    return (z, mu, sc), res


def kernel(context, eps, W1, b1, Wc, Wout, bout):
    (z, mu, sc), _ = run(context, eps, W1, b1, Wc, Wout, bout)
    return z, mu, sc
